# revision 2
# baseline (speedup 1.0000x reference)
"""DiffusionNCA on 8 Trainium2 NeuronCores.

Strategy (hardcoded for B=4, H=W=128, C=64, HIDDEN=256, steps=10):

- Pure data parallel over 8 shards: core = (batch b, image half). Each core
  owns 64 rows of one image and redundantly computes a 12-row halo band so
  that NO halo exchange is needed: the fake boundary error propagates only
  1 row/step (3x3 conv), and 12 > 10 steps, so owned rows stay exact.
- conv0/conv1/concat/fc0 are algebraically fused into 9 "tap" matmuls:
  h = sum_taps shift_tap(state) @ Wc[tap] + bias_total, where
  Wc[tap] = p0_w[tap] @ fc0_w[64:128] + p1_w[tap] @ fc0_w[128:192]
            (+ fc0_w[0:64] for the center tap). Reflect padding is
  materialized as 1-pixel pad rows/cols in the SBUF state buffer, so each
  tap is just a strided window of the same buffer (zero copies).
- BatchNorm batch stats: per-core partial sums (only over owned pixels,
  selected by a data-driven tile mask so the SPMD program is identical on
  every core) + a tiny [128,4] AllReduce per step.
- Dropout and fire-rate masks use jax threefry with a fixed seed(42) -> they
  are input-independent. They are precomputed on host (CPU), combined
  (keep & fire) and streamed to the device as u8 {0,1}; the 1/(1-p) dropout
  scale is folded into fc1_w.
- The whole 10-step rollout runs in ONE kernel launch; state lives in SBUF.
"""

import sys

for _p in ("/opt/trn_rl_repo", "/root/.axon_site/_ro/trn_rl_repo"):
    if _p not in sys.path:
        sys.path.append(_p)

import numpy as np

C_N = 64
HIDDEN = 256
FIRE_RATE = 0.5
DROP = 0.25
EPS = 1e-5
SLOPE = 0.01

B, H, W = 4, 128, 128
NCORES = 8
HALO = 12                  # >= steps, multiple of 4
ROWS = 64 + HALO           # 76 computed rows per core
TILES = ROWS // 4          # 19 row-tiles of 4 rows (512 pixels)
WP = W + 2                 # reflect-padded width
BROWS = ROWS + 2           # + top/bottom pad rows
NPIX_TOT = float(B * H * W)

_CACHE = {}


def _build_program(steps):
    import concourse.bacc as bacc
    import concourse.mybir as mybir
    from concourse import tile

    f32 = mybir.dt.float32
    u8 = mybir.dt.uint8
    AT = mybir.ActivationFunctionType
    ALU = mybir.AluOpType

    nc = bacc.Bacc("TRN2", target_bir_lowering=False, debug=False,
                   enable_asserts=True, num_devices=NCORES)

    state0 = nc.dram_tensor("state0", [C_N, BROWS, WP], f32, kind="ExternalInput")
    masks = nc.dram_tensor("masks", [steps, 2, TILES, 128, 512], u8, kind="ExternalInput")
    wtap = nc.dram_tensor("wtap", [C_N, 18 * 128], f32, kind="ExternalInput")
    fc1w = nc.dram_tensor("fc1w", [128, 128], f32, kind="ExternalInput")
    consts = nc.dram_tensor("consts", [128, 8], f32, kind="ExternalInput")
    omask = nc.dram_tensor("omask", [128, 4 * TILES], f32, kind="ExternalInput")
    out_t = nc.dram_tensor("out", [2, ROWS, W], f32, kind="ExternalOutput")

    TAPS = [(dy, dx) for dy in (-1, 0, 1) for dx in (-1, 0, 1)]

    with tile.TileContext(nc, num_cores=NCORES) as tc:
        with tc.tile_pool(name="const", bufs=1) as cpool, \
             tc.tile_pool(name="work", bufs=3) as wpool, \
             tc.tile_pool(name="small", bufs=2) as spool, \
             tc.tile_pool(name="psum", bufs=2, space="PSUM") as ppool, \
             tc.tile_pool(name="dram", bufs=2, space="DRAM") as dpool:

            w_sb = cpool.tile([C_N, 18 * 128], f32)
            fc1_sb = cpool.tile([128, 128], f32)
            c_sb = cpool.tile([128, 8], f32)
            om_sb = cpool.tile([128, 4 * TILES], f32)
            sA = cpool.tile([C_N, BROWS, WP], f32)
            sB = cpool.tile([C_N, BROWS, WP], f32)
            h_sb = cpool.tile([128, 2 * TILES * 512], f32)
            stats = cpool.tile([128, 4 * TILES], f32)

            nc.sync.dma_start(w_sb[:], wtap[:])
            nc.sync.dma_start(fc1_sb[:], fc1w[:])
            nc.sync.dma_start(c_sb[:], consts[:])
            nc.sync.dma_start(om_sb[:], omask[:])
            nc.sync.dma_start(sA[:], state0[:])

            for s in range(steps):
                cur = sA if s % 2 == 0 else sB
                nxt = sB if s % 2 == 0 else sA

                # ---- pass 1: fused conv+fc0 -> leaky relu -> h + stats ----
                for t in range(TILES):
                    for hh in range(2):
                        hp = ppool.tile([128, 512], f32, tag="hp")
                        for k, (dy, dx) in enumerate(TAPS):
                            nc.tensor.matmul(
                                hp[:],
                                w_sb[:, (k * 2 + hh) * 128:(k * 2 + hh + 1) * 128],
                                cur[:, 1 + 4 * t + dy: 5 + 4 * t + dy, 1 + dx: 129 + dx],
                                start=(k == 0), stop=(k == 8),
                            )
                        hsl = h_sb[:, (t * 2 + hh) * 512:(t * 2 + hh + 1) * 512]
                        nc.scalar.activation(hsl, hp[:], AT.Lrelu,
                                             bias=c_sb[:, hh:hh + 1], scale=1.0,
                                             alpha=SLOPE,
                                             accum_out=stats[:, hh * TILES + t: hh * TILES + t + 1])
                        sq = wpool.tile([128, 512], f32, tag="sq")
                        nc.scalar.activation(sq[:], hsl, AT.Square,
                                             accum_out=stats[:, (2 + hh) * TILES + t: (2 + hh) * TILES + t + 1])

                # ---- stats: mask non-owned tiles, reduce, AllReduce ----
                sm = spool.tile([128, 4 * TILES], f32, tag="sm")
                nc.vector.tensor_mul(sm[:], stats[:], om_sb[:])
                loc4 = spool.tile([128, 4], f32, tag="loc4")
                nc.vector.tensor_reduce(
                    loc4[:], sm[:].rearrange("p (g t) -> p g t", t=TILES),
                    axis=mybir.AxisListType.X, op=ALU.add)
                cin = dpool.tile([128, 4], f32, tag="cin")
                cout = dpool.tile([128, 4], f32, tag="cout")
                nc.sync.dma_start(cin[:], loc4[:])
                nc.gpsimd.collective_compute(
                    "AllReduce", ALU.add,
                    replica_groups=[list(range(NCORES))],
                    ins=[cin.opt()], outs=[cout.opt()])
                gs = spool.tile([128, 4], f32, tag="gs")
                nc.sync.dma_start(gs[:], cout[:])

                # mu = gs[:,0:2]/N ; msq = gs[:,2:4]/N ; var = msq - mu^2
                # scale = gamma*rsqrt(var+eps) ; shift = beta - mu*scale
                mug = spool.tile([128, 4], f32, tag="mug")
                nc.scalar.activation(mug[:], gs[:], AT.Identity, scale=c_sb[:, 7:8])
                musq = spool.tile([128, 2], f32, tag="musq")
                nc.vector.tensor_mul(musq[:], mug[:, 0:2], mug[:, 0:2])
                var = spool.tile([128, 2], f32, tag="var")
                nc.vector.tensor_sub(var[:], mug[:, 2:4], musq[:])
                std = spool.tile([128, 2], f32, tag="std")
                nc.scalar.activation(std[:], var[:], AT.Sqrt, bias=c_sb[:, 6:7])
                inv = spool.tile([128, 2], f32, tag="inv")
                nc.vector.reciprocal(inv[:], std[:])
                scl = spool.tile([128, 2], f32, tag="scl")
                nc.vector.tensor_mul(scl[:], c_sb[:, 2:4], inv[:])
                tmp2 = spool.tile([128, 2], f32, tag="tmp2")
                nc.vector.tensor_mul(tmp2[:], mug[:, 0:2], scl[:])
                shf = spool.tile([128, 2], f32, tag="shf")
                nc.vector.tensor_sub(shf[:], c_sb[:, 4:6], tmp2[:])

                # ---- pass 2: affine + dropout/fire mask + fc1 + update ----
                for t in range(TILES):
                    dxp = ppool.tile([C_N, 512], f32, tag="dxp")
                    for hh in range(2):
                        m_t = wpool.tile([128, 512], u8, tag="m")
                        nc.sync.dma_start(m_t[:], masks[s, hh, t])
                        hsl = h_sb[:, (t * 2 + hh) * 512:(t * 2 + hh + 1) * 512]
                        hn = wpool.tile([128, 512], f32, tag="hn")
                        nc.scalar.activation(hn[:], hsl, AT.Identity,
                                             bias=shf[:, hh:hh + 1], scale=scl[:, hh:hh + 1])
                        hf = wpool.tile([128, 512], f32, tag="hf")
                        nc.vector.tensor_tensor(out=hf[:], in0=hn[:], in1=m_t[:], op=ALU.mult)
                        nc.tensor.matmul(dxp[:], fc1_sb[:, hh * 64:(hh + 1) * 64], hf[:],
                                         start=(hh == 0), stop=(hh == 1))
                    nc.vector.tensor_add(
                        out=nxt[:, 1 + 4 * t: 5 + 4 * t, 1:129],
                        in0=dxp[:].rearrange("p (a b) -> p a b", b=128),
                        in1=cur[:, 1 + 4 * t: 5 + 4 * t, 1:129])

                # ---- reflect pads for next step ----
                nc.vector.tensor_copy(nxt[:, 1:BROWS - 1, 0:1], nxt[:, 1:BROWS - 1, 2:3])
                nc.vector.tensor_copy(nxt[:, 1:BROWS - 1, WP - 1:WP], nxt[:, 1:BROWS - 1, WP - 3:WP - 2])
                nc.vector.tensor_copy(nxt[:, 0:1, :], nxt[:, 2:3, :])
                nc.vector.tensor_copy(nxt[:, BROWS - 1:BROWS, :], nxt[:, BROWS - 3:BROWS - 2, :])

            final = sA if steps % 2 == 0 else sB
            nc.sync.dma_start(out_t[:], final[0:2, 1:BROWS - 1, 1:WP - 1])

    nc.compile()
    return nc


def _host_masks(steps):
    """Combined dropout-keep & fire masks, exactly matching the reference's
    threefry stream. Input-independent (seed 42)."""
    import jax

    cpu = jax.devices("cpu")[0]
    with jax.default_device(cpu):
        keys = jax.random.split(jax.random.key(42), steps)
        out = np.empty((steps, B, H, W, HIDDEN), np.uint8)
        for s in range(steps):
            k_drop, k_fire = jax.random.split(keys[s])
            keep = jax.random.bernoulli(k_drop, 1.0 - DROP, (B, H, W, HIDDEN))
            fire = jax.random.uniform(k_fire, (B, H, W, 1)) > FIRE_RATE
            out[s] = np.asarray(keep & fire, np.uint8)
    return out


def _prep_inputs(inputs, steps):
    x = np.asarray(inputs["x"], np.float32)
    t = np.asarray(inputs["t"], np.float32)
    p0w = np.asarray(inputs["p0_w"], np.float64)
    p0b = np.asarray(inputs["p0_b"], np.float64)
    p1w = np.asarray(inputs["p1_w"], np.float64)
    p1b = np.asarray(inputs["p1_b"], np.float64)
    fc0w = np.asarray(inputs["fc0_w"], np.float64)
    fc0b = np.asarray(inputs["fc0_b"], np.float64)
    fc1w = np.asarray(inputs["fc1_w"], np.float64)
    gamma = np.asarray(inputs["bn_gamma"], np.float32)
    beta = np.asarray(inputs["bn_beta"], np.float32)

    # fused conv+fc0 tap weights
    wtap_host = np.empty((C_N, 18 * 128), np.float32)
    for k in range(9):
        dy, dx = k // 3, k % 3
        wc = p0w[dy, dx] @ fc0w[C_N:2 * C_N] + p1w[dy, dx] @ fc0w[2 * C_N:3 * C_N]
        if dy == 1 and dx == 1:
            wc = wc + fc0w[0:C_N]
        wc = wc.astype(np.float32)
        for hh in range(2):
            wtap_host[:, (k * 2 + hh) * 128:(k * 2 + hh + 1) * 128] = wc[:, hh * 128:(hh + 1) * 128]

    bias_total = (p0b @ fc0w[C_N:2 * C_N] + p1b @ fc0w[2 * C_N:3 * C_N] + fc0b).astype(np.float32)

    fc1s = (fc1w / (1.0 - DROP)).astype(np.float32)
    fc1_host = np.zeros((128, 128), np.float32)
    for hh in range(2):
        fc1_host[:, hh * 64:(hh + 1) * 64] = fc1s[hh * 128:(hh + 1) * 128, :]

    consts = np.zeros((128, 8), np.float32)
    consts[:, 0] = bias_total[0:128]
    consts[:, 1] = bias_total[128:256]
    consts[:, 2] = gamma[0:128]
    consts[:, 3] = gamma[128:256]
    consts[:, 4] = beta[0:128]
    consts[:, 5] = beta[128:256]
    consts[:, 6] = EPS
    consts[:, 7] = 1.0 / NPIX_TOT

    # seed state
    st = np.zeros((B, H, W, C_N), np.float32)
    st[..., 1] = x[:, 0]
    lin = np.linspace(0.0, 1.0, H, dtype=np.float32)
    st[..., C_N - 2] = (lin[:, None] + lin[None, :]) * 0.5
    st[..., C_N - 1] = t[0]

    keep_eff = _host_masks(steps)

    in_maps = []
    for c in range(NCORES):
        b, half = c // 2, c % 2
        g0 = 0 if half == 0 else H - ROWS

        sl = st[b, g0:g0 + ROWS]                      # [76,128,64]
        p = np.concatenate([sl[1:2], sl, sl[ROWS - 2:ROWS - 1]], axis=0)   # pad rows
        p = np.concatenate([p[:, 1:2], p, p[:, W - 2:W - 1]], axis=1)      # pad cols
        state0_core = np.ascontiguousarray(p.transpose(2, 0, 1))           # [64,78,130]

        m = keep_eff[:, b, g0:g0 + ROWS]              # [S,76,128,256]
        m = m.reshape(steps, TILES, 4, W, 2, 128)
        m = np.ascontiguousarray(m.transpose(0, 4, 1, 5, 2, 3)).reshape(
            steps, 2, TILES, 128, 512)

        om = np.zeros((128, 4 * TILES), np.float32)
        owned = range(0, 16) if half == 0 else range(3, 19)
        for g in range(4):
            for tt in owned:
                om[:, g * TILES + tt] = 1.0

        in_maps.append({
            "state0": state0_core,
            "masks": m,
            "wtap": wtap_host,
            "fc1w": fc1_host,
            "consts": consts,
            "omask": om,
        })
    return in_maps


def _run(inputs, trace=False, **kw):
    from concourse.bass_utils import run_bass_kernel_spmd

    steps = int(np.asarray(inputs["steps"]))
    assert HALO >= steps, f"halo {HALO} < steps {steps}"
    if steps not in _CACHE:
        _CACHE[steps] = _build_program(steps)
    nc = _CACHE[steps]
    in_maps = _prep_inputs(inputs, steps)
    res = run_bass_kernel_spmd(nc, in_maps, core_ids=list(range(NCORES)),
                               trace=trace, **kw)

    out0 = np.zeros((B, 1, H, W), np.float32)
    out1 = np.zeros((B, H, W), np.float32)
    for c in range(NCORES):
        b, half = c // 2, c % 2
        o0 = 0 if half == 0 else HALO
        r = res.results[c]["out"]
        out0[b, 0, half * 64:(half + 1) * 64, :] = r[0, o0:o0 + 64, :]
        out1[b, half * 64:(half + 1) * 64, :] = r[1, o0:o0 + 64, :]
    return (out0, out1), res


def kernel(**inputs):
    (out0, out1), _ = _run(inputs)
    return out0, out1


# revision 3
# speedup vs baseline: 3.3051x; 3.3051x over previous
"""DiffusionNCA on 8 Trainium2 NeuronCores (v2: fp16 matmuls, paired taps).

Strategy (hardcoded for B=4, H=W=128, C=64, HIDDEN=256, steps=10):

- Pure data parallel over 8 shards: core = (batch b, image half). Each core
  owns 64 rows of one image plus a 12-row redundantly-computed halo band, so
  NO halo exchange is needed: a 3x3 conv propagates the fake-boundary error
  1 row/step and 12 > 10 steps, so owned rows stay exact.
- Bottom-half cores store their rows REVERSED (and receive dy-flipped tap
  weights) so every core's owned region is local rows 0..63 == row-tiles
  0..15: the SPMD program is identical, no per-core masking, and the BN
  AllReduce can launch after tile 15 while halo tiles 16..18 still compute.
- conv0/conv1/concat/fc0 are algebraically fused into 9 "tap" matmuls:
  h = sum_taps shift_tap(state) @ Wc[tap] + bias_total. Reflect padding is
  materialized as pad rows/cols of an fp16 state "mirror" whose upper 64
  partitions hold the +1-row-shifted image, so the 9 taps collapse to
  3 K=128 matmuls (dy in {-1,0} pairs) + 3 K=64 matmuls (dy=+1) per
  hidden-half, all reading strided windows of the same buffer (no copies).
- BatchNorm batch stats: per-core partial sums over owned pixels + a tiny
  [128,4] AllReduce per step. The affine is refactored as
  hf = (h + shift/scale)*mask,  dx = hf @ (diag(scale) @ fc1'), so the
  per-pixel affine fuses into the mask multiply (one DVE op) and scale
  folds into fc1 weights (two tiny per-step ops).
- Dropout/fire masks come from jax threefry with fixed seed(42) -> input-
  independent; precomputed on host, combined, streamed as u8 {0,1}; the
  1/(1-p) dropout scale is folded into fc1_w.
- fp32 master state (in-place residual add) + fp16 mirror for matmul reads;
  whole 10-step rollout in ONE kernel launch; state lives in SBUF.
"""

import sys

for _p in ("/opt/trn_rl_repo", "/root/.axon_site/_ro/trn_rl_repo"):
    if _p not in sys.path:
        sys.path.append(_p)

import numpy as np

C_N = 64
HIDDEN = 256
FIRE_RATE = 0.5
DROP = 0.25
EPS = 1e-5
SLOPE = 0.01

B, H, W = 4, 128, 128
NCORES = 8
HALO = 12                  # >= steps, multiple of 4
ROWS = 64 + HALO           # 76 computed rows per core
TILES = ROWS // 4          # 19 row-tiles of 4 rows (512 pixels)
OWNED_TILES = 16           # local tiles 0..15 are the owned 64 rows
WP = W + 2                 # reflect-padded width
BROWS = ROWS + 2           # + top/bottom pad rows
NPIX_TOT = float(B * H * W)

_CACHE = {}


def _build_program(steps):
    import concourse.bacc as bacc
    import concourse.mybir as mybir
    from concourse import tile

    f32 = mybir.dt.float32
    f16 = mybir.dt.float16
    u8 = mybir.dt.uint8
    AT = mybir.ActivationFunctionType
    ALU = mybir.AluOpType

    nc = bacc.Bacc("TRN2", target_bir_lowering=False, debug=False,
                   enable_asserts=True, num_devices=NCORES)

    mirror0 = nc.dram_tensor("mirror0", [128, BROWS, WP], f16, kind="ExternalInput")
    master0 = nc.dram_tensor("master0", [C_N, ROWS, W], f32, kind="ExternalInput")
    masks = nc.dram_tensor("masks", [steps, 2, TILES, 128, 512], u8, kind="ExternalInput")
    wtap = nc.dram_tensor("wtap", [128, 1536], f16, kind="ExternalInput")
    fc1w = nc.dram_tensor("fc1w", [128, 128], f16, kind="ExternalInput")
    consts = nc.dram_tensor("consts", [128, 8], f32, kind="ExternalInput")
    out_t = nc.dram_tensor("out", [2, ROWS, W], f32, kind="ExternalOutput")

    with tile.TileContext(nc, num_cores=NCORES) as tc:
        with tc.tile_pool(name="const", bufs=1) as cpool, \
             tc.tile_pool(name="work", bufs=3) as wpool, \
             tc.tile_pool(name="mpool", bufs=8) as mpool, \
             tc.tile_pool(name="small", bufs=2) as spool, \
             tc.tile_pool(name="psum", bufs=2, space="PSUM") as ppool, \
             tc.tile_pool(name="dram", bufs=2, space="DRAM") as dpool:

            w_sb = cpool.tile([128, 1536], f16)
            fc1_sb = cpool.tile([128, 128], f16)
            c_sb = cpool.tile([128, 8], f32)
            mirror = cpool.tile([128, BROWS, WP], f16)
            master = cpool.tile([C_N, ROWS, W], f32)
            h_sb = cpool.tile([128, 2 * TILES * 512], f16)
            stats = cpool.tile([128, 4 * OWNED_TILES], f32)

            nc.sync.dma_start(w_sb[:], wtap[:])
            nc.sync.dma_start(fc1_sb[:], fc1w[:])
            nc.sync.dma_start(c_sb[:], consts[:])
            nc.sync.dma_start(mirror[:], mirror0[:])
            nc.sync.dma_start(master[:], master0[:])

            for s in range(steps):

                def pass1_tile(t, accum):
                    for hh in range(2):
                        hp = ppool.tile([128, 512], f32, tag="hp")
                        for i in range(3):          # dx = -1, 0, 1
                            nc.tensor.matmul(      # dy=-1 (lower) + dy=0 (upper)
                                hp[:],
                                w_sb[:, (i * 2 + hh) * 128:(i * 2 + hh + 1) * 128],
                                mirror[:, 4 * t: 4 * t + 4, i: i + 128],
                                start=(i == 0), stop=False)
                        for i in range(3):          # dy=+1 via upper half
                            nc.tensor.matmul(
                                hp[:],
                                w_sb[64:128, 768 + (i * 2 + hh) * 128: 768 + (i * 2 + hh + 1) * 128],
                                mirror[64:128, 4 * t + 1: 4 * t + 5, i: i + 128],
                                start=False, stop=(i == 2))
                        hsl = h_sb[:, (t * 2 + hh) * 512:(t * 2 + hh + 1) * 512]
                        if accum:
                            nc.scalar.activation(
                                hsl, hp[:], AT.Lrelu, bias=c_sb[:, hh:hh + 1],
                                scale=1.0, alpha=SLOPE,
                                accum_out=stats[:, hh * OWNED_TILES + t: hh * OWNED_TILES + t + 1])
                            sq = wpool.tile([128, 512], f16, tag="sq")
                            nc.vector.scalar_tensor_tensor(
                                out=sq[:], in0=hsl, scalar=0.0, in1=hsl,
                                op0=ALU.add, op1=ALU.mult,
                                accum_out=stats[:, (2 + hh) * OWNED_TILES + t: (2 + hh) * OWNED_TILES + t + 1])
                        else:
                            nc.scalar.activation(hsl, hp[:], AT.Lrelu,
                                                 bias=c_sb[:, hh:hh + 1],
                                                 scale=1.0, alpha=SLOPE)

                # ---- pass 1 on owned tiles, then AR, then halo tiles ----
                for t in range(OWNED_TILES):
                    pass1_tile(t, True)

                loc4 = spool.tile([128, 4], f32, tag="loc4")
                nc.vector.tensor_reduce(
                    loc4[:], stats[:].rearrange("p (g t) -> p g t", t=OWNED_TILES),
                    axis=mybir.AxisListType.X, op=ALU.add)
                cin = dpool.tile([128, 4], f32, tag="cin")
                cout = dpool.tile([128, 4], f32, tag="cout")
                nc.sync.dma_start(cin[:], loc4[:])
                nc.gpsimd.collective_compute(
                    "AllReduce", ALU.add,
                    replica_groups=[list(range(NCORES))],
                    ins=[cin.opt()], outs=[cout.opt()])
                gs = spool.tile([128, 4], f32, tag="gs")
                nc.sync.dma_start(gs[:], cout[:])

                for t in range(OWNED_TILES, TILES):
                    pass1_tile(t, False)

                # ---- BN coefficients ----
                mug = spool.tile([128, 4], f32, tag="mug")
                nc.vector.tensor_scalar_mul(mug[:], gs[:], 1.0 / NPIX_TOT)
                musq = spool.tile([128, 2], f32, tag="musq")
                nc.vector.tensor_mul(musq[:], mug[:, 0:2], mug[:, 0:2])
                var = spool.tile([128, 2], f32, tag="var")
                nc.vector.tensor_sub(var[:], mug[:, 2:4], musq[:])
                std = spool.tile([128, 2], f32, tag="std")
                nc.scalar.activation(std[:], var[:], AT.Sqrt, bias=c_sb[:, 6:7])
                inv = spool.tile([128, 2], f32, tag="inv")
                nc.vector.reciprocal(inv[:], std[:])
                scl = spool.tile([128, 2], f32, tag="scl")
                nc.vector.tensor_mul(scl[:], c_sb[:, 2:4], inv[:])
                rb = spool.tile([128, 2], f32, tag="rb")
                nc.vector.reciprocal(rb[:], scl[:])
                sh2a = spool.tile([128, 2], f32, tag="sh2a")
                nc.vector.tensor_mul(sh2a[:], c_sb[:, 4:6], rb[:])
                sh2 = spool.tile([128, 2], f32, tag="sh2")
                nc.vector.tensor_sub(sh2[:], sh2a[:], mug[:, 0:2])
                fc1p = spool.tile([128, 128], f16, tag="fc1p")
                nc.vector.tensor_scalar_mul(fc1p[:, 0:64], fc1_sb[:, 0:64], scl[:, 0:1])
                nc.vector.tensor_scalar_mul(fc1p[:, 64:128], fc1_sb[:, 64:128], scl[:, 1:2])

                # ---- pass 2: hf = (h + sh2)*mask ; dx = hf @ fc1p ; update ----
                for t in range(TILES):
                    dxp = ppool.tile([C_N, 512], f32, tag="dxp")
                    for hh in range(2):
                        m_t = mpool.tile([128, 512], u8, tag="m")
                        nc.sync.dma_start(m_t[:], masks[s, hh, t])
                        hf = wpool.tile([128, 512], f16, tag="hf")
                        nc.vector.scalar_tensor_tensor(
                            out=hf[:], in0=h_sb[:, (t * 2 + hh) * 512:(t * 2 + hh + 1) * 512],
                            scalar=sh2[:, hh:hh + 1], in1=m_t[:],
                            op0=ALU.add, op1=ALU.mult)
                        nc.tensor.matmul(dxp[:], fc1p[:, hh * 64:(hh + 1) * 64], hf[:],
                                         start=(hh == 0), stop=(hh == 1))
                    nc.vector.tensor_add(
                        out=master[:, 4 * t: 4 * t + 4, :],
                        in0=dxp[:].rearrange("p (a b) -> p a b", b=128),
                        in1=master[:, 4 * t: 4 * t + 4, :])
                    nc.gpsimd.tensor_copy(mirror[0:C_N, 1 + 4 * t: 5 + 4 * t, 1:129],
                                          master[:, 4 * t: 4 * t + 4, :])
                    nc.gpsimd.tensor_copy(mirror[C_N:128, 4 * t: 4 * t + 4, 1:129],
                                          master[:, 4 * t: 4 * t + 4, :])

                # ---- reflect pads on the mirror ----
                nc.gpsimd.tensor_copy(mirror[0:C_N, 0:1, 1:129], mirror[0:C_N, 2:3, 1:129])
                nc.gpsimd.tensor_copy(mirror[0:C_N, BROWS - 1:BROWS, 1:129],
                                      mirror[0:C_N, BROWS - 3:BROWS - 2, 1:129])
                nc.gpsimd.tensor_copy(mirror[C_N:128, BROWS - 2:BROWS - 1, 1:129],
                                      mirror[C_N:128, BROWS - 4:BROWS - 3, 1:129])
                nc.gpsimd.tensor_copy(mirror[:, :, 0:1], mirror[:, :, 2:3])
                nc.gpsimd.tensor_copy(mirror[:, :, WP - 1:WP], mirror[:, :, WP - 3:WP - 2])

            nc.sync.dma_start(out_t[:], master[0:2, :, :])

    nc.compile()
    return nc


def _host_masks(steps):
    """Combined dropout-keep & fire masks, exactly matching the reference's
    threefry stream. Input-independent (seed 42)."""
    import jax

    cpu = jax.devices("cpu")[0]
    with jax.default_device(cpu):
        keys = jax.random.split(jax.random.key(42), steps)
        out = np.empty((steps, B, H, W, HIDDEN), np.uint8)
        for s in range(steps):
            k_drop, k_fire = jax.random.split(keys[s])
            keep = jax.random.bernoulli(k_drop, 1.0 - DROP, (B, H, W, HIDDEN))
            fire = jax.random.uniform(k_fire, (B, H, W, 1)) > FIRE_RATE
            out[s] = np.asarray(keep & fire, np.uint8)
    return out


def _prep_inputs(inputs, steps):
    x = np.asarray(inputs["x"], np.float32)
    t = np.asarray(inputs["t"], np.float32)
    p0w = np.asarray(inputs["p0_w"], np.float64)
    p0b = np.asarray(inputs["p0_b"], np.float64)
    p1w = np.asarray(inputs["p1_w"], np.float64)
    p1b = np.asarray(inputs["p1_b"], np.float64)
    fc0w = np.asarray(inputs["fc0_w"], np.float64)
    fc0b = np.asarray(inputs["fc0_b"], np.float64)
    fc1w = np.asarray(inputs["fc1_w"], np.float64)
    gamma = np.asarray(inputs["bn_gamma"], np.float32)
    beta = np.asarray(inputs["bn_beta"], np.float32)

    # fused conv+fc0 tap weights, [3(dy),3(dx),64,256]
    Wc = np.zeros((3, 3, C_N, HIDDEN), np.float32)
    for r in range(3):
        for c in range(3):
            wc = p0w[r, c] @ fc0w[C_N:2 * C_N] + p1w[r, c] @ fc0w[2 * C_N:]
            if r == 1 and c == 1:
                wc = wc + fc0w[0:C_N]
            Wc[r, c] = wc.astype(np.float32)
    bias_total = (p0b @ fc0w[C_N:2 * C_N] + p1b @ fc0w[2 * C_N:] + fc0b).astype(np.float32)

    # wtap layout per flip variant:
    #   pair block (i=dx_idx, hh): cols (i*2+hh)*128, rows 0:64 = W(dy=-1,dx),
    #                              rows 64:128 = W(dy=0,dx)
    #   solo block: cols 768+(i*2+hh)*128, rows 64:128 = W(dy=+1,dx)
    def build_wtap(flip):
        wt = np.zeros((128, 1536), np.float16)
        for i in range(3):          # dx index: dx = i-1
            for hh in range(2):
                cdx = i             # column index into Wc
                wm1 = Wc[2 if flip else 0, cdx]   # local dy=-1
                w0 = Wc[1, cdx]                   # local dy=0
                wp1 = Wc[0 if flip else 2, cdx]   # local dy=+1
                col = (i * 2 + hh) * 128
                wt[0:C_N, col:col + 128] = wm1[:, hh * 128:(hh + 1) * 128]
                wt[C_N:128, col:col + 128] = w0[:, hh * 128:(hh + 1) * 128]
                scol = 768 + (i * 2 + hh) * 128
                wt[C_N:128, scol:scol + 128] = wp1[:, hh * 128:(hh + 1) * 128]
        return wt

    wtap_by_flip = [build_wtap(False), build_wtap(True)]

    fc1s = (fc1w / (1.0 - DROP))
    fc1_host = np.zeros((128, 128), np.float16)
    for hh in range(2):
        fc1_host[:, hh * 64:(hh + 1) * 64] = fc1s[hh * 128:(hh + 1) * 128, :].astype(np.float16)

    consts = np.zeros((128, 8), np.float32)
    consts[:, 0] = bias_total[0:128]
    consts[:, 1] = bias_total[128:256]
    consts[:, 2] = gamma[0:128]
    consts[:, 3] = gamma[128:256]
    consts[:, 4] = beta[0:128]
    consts[:, 5] = beta[128:256]
    consts[:, 6] = EPS
    consts[:, 7] = 1.0 / NPIX_TOT

    # seed state [B,H,W,C]
    st = np.zeros((B, H, W, C_N), np.float32)
    st[..., 1] = x[:, 0]
    lin = np.linspace(0.0, 1.0, H, dtype=np.float32)
    st[..., C_N - 2] = (lin[:, None] + lin[None, :]) * 0.5
    st[..., C_N - 1] = t[0]

    keep_eff = _host_masks(steps)

    in_maps = []
    for c in range(NCORES):
        b, half = c // 2, c % 2
        flip = half == 1

        if not flip:
            sl = st[b, 0:ROWS]                       # [76,128,64] local==global
            msl = keep_eff[:, b, 0:ROWS]             # [S,76,128,256]
        else:
            sl = st[b, H - ROWS:H][::-1]             # local i = global 127-i
            msl = keep_eff[:, b, H - ROWS:H][:, ::-1]

        master0 = np.ascontiguousarray(sl.transpose(2, 0, 1))          # [64,76,128] f32

        p = np.concatenate([sl[1:2], sl, sl[ROWS - 2:ROWS - 1]], axis=0)
        p = np.concatenate([p[:, 1:2], p, p[:, W - 2:W - 1]], axis=1)  # [78,130,64]
        low = p.transpose(2, 0, 1).astype(np.float16)                  # [64,78,130]
        up = np.zeros_like(low)
        up[:, 0:BROWS - 1] = low[:, 1:BROWS]
        mirror0 = np.ascontiguousarray(np.concatenate([low, up], axis=0))  # [128,78,130]

        m = msl.reshape(steps, TILES, 4, W, 2, 128)
        m = np.ascontiguousarray(m.transpose(0, 4, 1, 5, 2, 3)).reshape(
            steps, 2, TILES, 128, 512)

        in_maps.append({
            "mirror0": mirror0,
            "master0": master0,
            "masks": m,
            "wtap": wtap_by_flip[flip],
            "fc1w": fc1_host,
            "consts": consts,
        })
    return in_maps


def _run(inputs, trace=False, **kw):
    from concourse.bass_utils import run_bass_kernel_spmd

    steps = int(np.asarray(inputs["steps"]))
    assert HALO >= steps, f"halo {HALO} < steps {steps}"
    if steps not in _CACHE:
        _CACHE[steps] = _build_program(steps)
    nc = _CACHE[steps]
    in_maps = _prep_inputs(inputs, steps)
    res = run_bass_kernel_spmd(nc, in_maps, core_ids=list(range(NCORES)),
                               trace=trace, **kw)

    out0 = np.zeros((B, 1, H, W), np.float32)
    out1 = np.zeros((B, H, W), np.float32)
    for c in range(NCORES):
        b, half = c // 2, c % 2
        r = res.results[c]["out"][:, 0:64, :]        # owned local rows 0..63
        if half == 1:
            r = r[:, ::-1, :]                        # un-flip
        rows = slice(half * 64, (half + 1) * 64)
        out0[b, 0, rows, :] = r[0]
        out1[b, rows, :] = r[1]
    return (out0, out1), res


def kernel(**inputs):
    (out0, out1), _ = _run(inputs)
    return out0, out1


# revision 4
# speedup vs baseline: 4.1710x; 1.2620x over previous
"""DiffusionNCA on 8 Trainium2 NeuronCores (v3).

Strategy (hardcoded for B=4, H=W=128, C=64, HIDDEN=256, steps=10):

- Pure data parallel over 8 shards: core = (batch b, image half). Each core
  owns 64 rows of one image plus a 12-row redundantly-computed halo band, so
  NO halo exchange is needed: a 3x3 conv propagates the fake-boundary error
  1 row/step and 12 > 10 steps, so owned rows stay exact.
- Bottom-half cores store their rows REVERSED (and receive dy-flipped tap
  weights) so every core's owned region is local rows 0..63 == row-tiles
  0..15: the SPMD program is identical on all cores, and the BN AllReduce
  launches right after tile 15 while halo tiles 16..18 still compute.
- conv0/conv1/concat/fc0 are algebraically fused into 9 "tap" matmuls:
  h = sum_taps shift_tap(state) @ Wc[tap] + bias_total. Reflect padding is
  materialized as pad rows/cols of the fp16 state buffer whose upper 64
  partitions hold the +1-row-shifted image, so the 9 taps collapse to
  3 K=128 matmuls (dy in {-1,0} pairs) + 3 K=64 matmuls (dy=+1) per
  hidden-half, all reading strided windows of the same buffer (no copies).
- BatchNorm batch stats: per-core partial sums over owned pixels + a tiny
  [128,4] AllReduce per step. The BN affine is algebraically folded away:
      dx = ((h + sh2) * mask) @ (diag(scl) @ fc1)        sh2 = shift/scl
         = (h * mask) @ fc1p + mask @ G,
  with fc1p = diag(scl) @ fc1 and G = diag(sh2) @ fc1p computed per step in
  four tiny ops, so per pixel only a plain fp16 multiply remains on DVE.
- Dropout/fire masks come from jax threefry with fixed seed(42) -> input-
  independent; precomputed on host, combined, streamed as fp16 {0,1}; the
  1/(1-p) dropout scale is folded into fc1_w.
- State lives entirely in SBUF as fp16 for the whole 10-step rollout in a
  single kernel launch (in-place residual updates; lower+upper written from
  the same PSUM tile so both copies stay bit-identical).
"""

import sys

for _p in ("/opt/trn_rl_repo", "/root/.axon_site/_ro/trn_rl_repo"):
    if _p not in sys.path:
        sys.path.append(_p)

import numpy as np

C_N = 64
HIDDEN = 256
FIRE_RATE = 0.5
DROP = 0.25
EPS = 1e-5
SLOPE = 0.01

B, H, W = 4, 128, 128
NCORES = 8
HALO = 12                  # >= steps, multiple of 4
ROWS = 64 + HALO           # 76 computed rows per core
TILES = ROWS // 4          # 19 row-tiles of 4 rows (512 pixels)
OWNED_TILES = 16           # local tiles 0..15 are the owned 64 rows
WP = W + 2                 # reflect-padded width
BROWS = ROWS + 2           # + top/bottom pad rows
NPIX_TOT = float(B * H * W)

_CACHE = {}


def _build_program(steps):
    import concourse.bacc as bacc
    import concourse.mybir as mybir
    from concourse import tile

    f32 = mybir.dt.float32
    f16 = mybir.dt.float16
    AT = mybir.ActivationFunctionType
    ALU = mybir.AluOpType

    nc = bacc.Bacc("TRN2", target_bir_lowering=False, debug=False,
                   enable_asserts=True, num_devices=NCORES)

    mirror0 = nc.dram_tensor("mirror0", [128, BROWS, WP], f16, kind="ExternalInput")
    masks = nc.dram_tensor("masks", [steps, 2, TILES, 128, 512], f16, kind="ExternalInput")
    wtap = nc.dram_tensor("wtap", [128, 1536], f16, kind="ExternalInput")
    fc1w = nc.dram_tensor("fc1w", [128, 128], f16, kind="ExternalInput")
    consts = nc.dram_tensor("consts", [128, 8], f32, kind="ExternalInput")
    out_t = nc.dram_tensor("out", [2, ROWS, W], f16, kind="ExternalOutput")

    with tile.TileContext(nc, num_cores=NCORES) as tc:
        with tc.tile_pool(name="const", bufs=1) as cpool, \
             tc.tile_pool(name="work", bufs=3) as wpool, \
             tc.tile_pool(name="mpool", bufs=16) as mpool, \
             tc.tile_pool(name="small", bufs=2) as spool, \
             tc.tile_pool(name="psum", bufs=2, space="PSUM") as ppool, \
             tc.tile_pool(name="dram", bufs=2, space="DRAM") as dpool:

            w_sb = cpool.tile([128, 1536], f16)
            fc1_sb = cpool.tile([128, 128], f16)
            c_sb = cpool.tile([128, 8], f32)
            mirror = cpool.tile([128, BROWS, WP], f16)
            h_sb = cpool.tile([128, 2 * TILES * 512], f16)
            stats = cpool.tile([128, 4 * OWNED_TILES], f32)

            nc.sync.dma_start(w_sb[:], wtap[:])
            nc.sync.dma_start(fc1_sb[:], fc1w[:])
            nc.sync.dma_start(c_sb[:], consts[:])
            nc.sync.dma_start(mirror[:], mirror0[:])

            for s in range(steps):

                def pass1_tile(t, accum):
                    for hh in range(2):
                        hp = ppool.tile([128, 512], f32, tag="hp")
                        for i in range(3):          # dx = -1, 0, 1
                            nc.tensor.matmul(      # dy=-1 (lower) + dy=0 (upper)
                                hp[:],
                                w_sb[:, (i * 2 + hh) * 128:(i * 2 + hh + 1) * 128],
                                mirror[:, 4 * t: 4 * t + 4, i: i + 128],
                                start=(i == 0), stop=False)
                        for i in range(3):          # dy=+1 via upper half
                            nc.tensor.matmul(
                                hp[:],
                                w_sb[64:128, 768 + (i * 2 + hh) * 128: 768 + (i * 2 + hh + 1) * 128],
                                mirror[64:128, 4 * t + 1: 4 * t + 5, i: i + 128],
                                start=False, stop=(i == 2))
                        hsl = h_sb[:, (t * 2 + hh) * 512:(t * 2 + hh + 1) * 512]
                        if accum:
                            nc.scalar.activation(
                                hsl, hp[:], AT.Lrelu, bias=c_sb[:, hh:hh + 1],
                                scale=1.0, alpha=SLOPE,
                                accum_out=stats[:, hh * OWNED_TILES + t: hh * OWNED_TILES + t + 1])
                            sq = wpool.tile([128, 512], f16, tag="sq")
                            nc.scalar.activation(
                                sq[:], hsl, AT.Square,
                                accum_out=stats[:, (2 + hh) * OWNED_TILES + t: (2 + hh) * OWNED_TILES + t + 1])
                        else:
                            nc.scalar.activation(hsl, hp[:], AT.Lrelu,
                                                 bias=c_sb[:, hh:hh + 1],
                                                 scale=1.0, alpha=SLOPE)

                # ---- pass 1 on owned tiles, then AR, then halo tiles ----
                for t in range(OWNED_TILES):
                    pass1_tile(t, True)

                loc4 = spool.tile([128, 4], f32, tag="loc4")
                nc.vector.tensor_reduce(
                    loc4[:], stats[:].rearrange("p (g t) -> p g t", t=OWNED_TILES),
                    axis=mybir.AxisListType.X, op=ALU.add)
                cin = dpool.tile([128, 4], f32, tag="cin")
                cout = dpool.tile([128, 4], f32, tag="cout")
                nc.sync.dma_start(cin[:], loc4[:])
                nc.gpsimd.collective_compute(
                    "AllReduce", ALU.add,
                    replica_groups=[list(range(NCORES))],
                    ins=[cin.opt()], outs=[cout.opt()])
                gs = spool.tile([128, 4], f32, tag="gs")
                nc.sync.dma_start(gs[:], cout[:])

                for t in range(OWNED_TILES, TILES):
                    pass1_tile(t, False)

                # ---- BN coefficients (all DVE except the Sqrt) ----
                mug = spool.tile([128, 4], f32, tag="mug")
                nc.vector.tensor_scalar_mul(mug[:], gs[:], 1.0 / NPIX_TOT)
                musq = spool.tile([128, 2], f32, tag="musq")
                nc.vector.tensor_mul(musq[:], mug[:, 0:2], mug[:, 0:2])
                var = spool.tile([128, 2], f32, tag="var")
                nc.vector.tensor_sub(var[:], mug[:, 2:4], musq[:])
                std = spool.tile([128, 2], f32, tag="std")
                nc.scalar.activation(std[:], var[:], AT.Sqrt, bias=c_sb[:, 6:7])
                inv = spool.tile([128, 2], f32, tag="inv")
                nc.vector.reciprocal(inv[:], std[:])
                scl = spool.tile([128, 2], f32, tag="scl")
                nc.vector.tensor_mul(scl[:], c_sb[:, 2:4], inv[:])
                rb = spool.tile([128, 2], f32, tag="rb")
                nc.vector.reciprocal(rb[:], scl[:])
                sh2a = spool.tile([128, 2], f32, tag="sh2a")
                nc.vector.tensor_mul(sh2a[:], c_sb[:, 4:6], rb[:])
                sh2 = spool.tile([128, 2], f32, tag="sh2")
                nc.vector.tensor_sub(sh2[:], sh2a[:], mug[:, 0:2])
                fc1p = spool.tile([128, 128], f16, tag="fc1p")
                nc.vector.tensor_scalar_mul(fc1p[:, 0:64], fc1_sb[:, 0:64], scl[:, 0:1])
                nc.vector.tensor_scalar_mul(fc1p[:, 64:128], fc1_sb[:, 64:128], scl[:, 1:2])
                gmat = spool.tile([128, 128], f16, tag="gmat")
                nc.vector.tensor_scalar_mul(gmat[:, 0:64], fc1p[:, 0:64], sh2[:, 0:1])
                nc.vector.tensor_scalar_mul(gmat[:, 64:128], fc1p[:, 64:128], sh2[:, 1:2])

                # ---- pass 2: dx = (h*mask) @ fc1p + mask @ G ; state += dx ----
                for t in range(TILES):
                    dxp = ppool.tile([C_N, 512], f32, tag="dxp")
                    for hh in range(2):
                        m_t = mpool.tile([128, 512], f16, tag="m")
                        nc.sync.dma_start(m_t[:], masks[s, hh, t])
                        hf = wpool.tile([128, 512], f16, tag="hf")
                        nc.vector.tensor_mul(
                            hf[:], h_sb[:, (t * 2 + hh) * 512:(t * 2 + hh + 1) * 512], m_t[:])
                        nc.tensor.matmul(dxp[:], fc1p[:, hh * 64:(hh + 1) * 64], hf[:],
                                         start=(hh == 0), stop=False)
                        nc.tensor.matmul(dxp[:], gmat[:, hh * 64:(hh + 1) * 64], m_t[:],
                                         start=False, stop=(hh == 1))
                    # upper first (reads the pre-update lower rows), then lower
                    nc.vector.tensor_add(
                        out=mirror[C_N:128, 4 * t: 4 * t + 4, 1:129],
                        in0=dxp[:].rearrange("p (a b) -> p a b", b=128),
                        in1=mirror[0:C_N, 1 + 4 * t: 5 + 4 * t, 1:129])
                    nc.vector.tensor_add(
                        out=mirror[0:C_N, 1 + 4 * t: 5 + 4 * t, 1:129],
                        in0=dxp[:].rearrange("p (a b) -> p a b", b=128),
                        in1=mirror[0:C_N, 1 + 4 * t: 5 + 4 * t, 1:129])

                # ---- reflect pads ----
                nc.gpsimd.tensor_copy(mirror[0:C_N, 0:1, 1:129], mirror[0:C_N, 2:3, 1:129])
                nc.gpsimd.tensor_copy(mirror[0:C_N, BROWS - 1:BROWS, 1:129],
                                      mirror[0:C_N, BROWS - 3:BROWS - 2, 1:129])
                nc.gpsimd.tensor_copy(mirror[C_N:128, BROWS - 2:BROWS - 1, 1:129],
                                      mirror[C_N:128, BROWS - 4:BROWS - 3, 1:129])
                nc.gpsimd.tensor_copy(mirror[:, :, 0:1], mirror[:, :, 2:3])
                nc.gpsimd.tensor_copy(mirror[:, :, WP - 1:WP], mirror[:, :, WP - 3:WP - 2])

            nc.sync.dma_start(out_t[:], mirror[0:2, 1:BROWS - 1, 1:WP - 1])

    nc.compile()
    return nc


def _host_masks(steps):
    """Combined dropout-keep & fire masks, exactly matching the reference's
    threefry stream. Input-independent (seed 42)."""
    import jax

    cpu = jax.devices("cpu")[0]
    with jax.default_device(cpu):
        keys = jax.random.split(jax.random.key(42), steps)
        out = np.empty((steps, B, H, W, HIDDEN), np.uint8)
        for s in range(steps):
            k_drop, k_fire = jax.random.split(keys[s])
            keep = jax.random.bernoulli(k_drop, 1.0 - DROP, (B, H, W, HIDDEN))
            fire = jax.random.uniform(k_fire, (B, H, W, 1)) > FIRE_RATE
            out[s] = np.asarray(keep & fire, np.uint8)
    return out


def _prep_inputs(inputs, steps):
    x = np.asarray(inputs["x"], np.float32)
    t = np.asarray(inputs["t"], np.float32)
    p0w = np.asarray(inputs["p0_w"], np.float64)
    p0b = np.asarray(inputs["p0_b"], np.float64)
    p1w = np.asarray(inputs["p1_w"], np.float64)
    p1b = np.asarray(inputs["p1_b"], np.float64)
    fc0w = np.asarray(inputs["fc0_w"], np.float64)
    fc0b = np.asarray(inputs["fc0_b"], np.float64)
    fc1w = np.asarray(inputs["fc1_w"], np.float64)
    gamma = np.asarray(inputs["bn_gamma"], np.float32)
    beta = np.asarray(inputs["bn_beta"], np.float32)

    # fused conv+fc0 tap weights, [3(dy),3(dx),64,256]
    Wc = np.zeros((3, 3, C_N, HIDDEN), np.float32)
    for r in range(3):
        for c in range(3):
            wc = p0w[r, c] @ fc0w[C_N:2 * C_N] + p1w[r, c] @ fc0w[2 * C_N:]
            if r == 1 and c == 1:
                wc = wc + fc0w[0:C_N]
            Wc[r, c] = wc.astype(np.float32)
    bias_total = (p0b @ fc0w[C_N:2 * C_N] + p1b @ fc0w[2 * C_N:] + fc0b).astype(np.float32)

    def build_wtap(flip):
        wt = np.zeros((128, 1536), np.float16)
        for i in range(3):          # dx index
            for hh in range(2):
                wm1 = Wc[2 if flip else 0, i]     # local dy=-1
                w0 = Wc[1, i]                     # local dy=0
                wp1 = Wc[0 if flip else 2, i]     # local dy=+1
                col = (i * 2 + hh) * 128
                wt[0:C_N, col:col + 128] = wm1[:, hh * 128:(hh + 1) * 128]
                wt[C_N:128, col:col + 128] = w0[:, hh * 128:(hh + 1) * 128]
                scol = 768 + (i * 2 + hh) * 128
                wt[C_N:128, scol:scol + 128] = wp1[:, hh * 128:(hh + 1) * 128]
        return wt

    wtap_by_flip = [build_wtap(False), build_wtap(True)]

    fc1s = fc1w / (1.0 - DROP)
    fc1_host = np.zeros((128, 128), np.float16)
    for hh in range(2):
        fc1_host[:, hh * 64:(hh + 1) * 64] = fc1s[hh * 128:(hh + 1) * 128, :].astype(np.float16)

    consts = np.zeros((128, 8), np.float32)
    consts[:, 0] = bias_total[0:128]
    consts[:, 1] = bias_total[128:256]
    consts[:, 2] = gamma[0:128]
    consts[:, 3] = gamma[128:256]
    consts[:, 4] = beta[0:128]
    consts[:, 5] = beta[128:256]
    consts[:, 6] = EPS
    consts[:, 7] = 1.0 / NPIX_TOT

    # seed state [B,H,W,C]
    st = np.zeros((B, H, W, C_N), np.float32)
    st[..., 1] = x[:, 0]
    lin = np.linspace(0.0, 1.0, H, dtype=np.float32)
    st[..., C_N - 2] = (lin[:, None] + lin[None, :]) * 0.5
    st[..., C_N - 1] = t[0]

    keep_eff = _host_masks(steps)

    in_maps = []
    for c in range(NCORES):
        b, half = c // 2, c % 2
        flip = half == 1

        if not flip:
            sl = st[b, 0:ROWS]                       # [76,128,64] local==global
            msl = keep_eff[:, b, 0:ROWS]
        else:
            sl = st[b, H - ROWS:H][::-1]             # local i = global 127-i
            msl = keep_eff[:, b, H - ROWS:H][:, ::-1]

        p = np.concatenate([sl[1:2], sl, sl[ROWS - 2:ROWS - 1]], axis=0)
        p = np.concatenate([p[:, 1:2], p, p[:, W - 2:W - 1]], axis=1)  # [78,130,64]
        low = p.transpose(2, 0, 1).astype(np.float16)                  # [64,78,130]
        up = np.zeros_like(low)
        up[:, 0:BROWS - 1] = low[:, 1:BROWS]
        mirror0 = np.ascontiguousarray(np.concatenate([low, up], axis=0))

        m = msl.reshape(steps, TILES, 4, W, 2, 128)
        m = np.ascontiguousarray(
            m.transpose(0, 4, 1, 5, 2, 3)).reshape(steps, 2, TILES, 128, 512).astype(np.float16)

        in_maps.append({
            "mirror0": mirror0,
            "masks": m,
            "wtap": wtap_by_flip[flip],
            "fc1w": fc1_host,
            "consts": consts,
        })
    return in_maps


def _run(inputs, trace=False, **kw):
    from concourse.bass_utils import run_bass_kernel_spmd

    steps = int(np.asarray(inputs["steps"]))
    assert HALO >= steps, f"halo {HALO} < steps {steps}"
    if steps not in _CACHE:
        _CACHE[steps] = _build_program(steps)
    nc = _CACHE[steps]
    in_maps = _prep_inputs(inputs, steps)
    res = run_bass_kernel_spmd(nc, in_maps, core_ids=list(range(NCORES)),
                               trace=trace, **kw)

    out0 = np.zeros((B, 1, H, W), np.float32)
    out1 = np.zeros((B, H, W), np.float32)
    for c in range(NCORES):
        b, half = c // 2, c % 2
        r = res.results[c]["out"][:, 0:64, :].astype(np.float32)
        if half == 1:
            r = r[:, ::-1, :]
        rows = slice(half * 64, (half + 1) * 64)
        out0[b, 0, rows, :] = r[0]
        out1[b, rows, :] = r[1]
    return (out0, out1), res


def kernel(**inputs):
    (out0, out1), _ = _run(inputs)
    return out0, out1


# revision 5
# speedup vs baseline: 5.3162x; 1.2746x over previous
"""DiffusionNCA on 8 Trainium2 NeuronCores (v3).

Strategy (hardcoded for B=4, H=W=128, C=64, HIDDEN=256, steps=10):

- Pure data parallel over 8 shards: core = (batch b, image half). Each core
  owns 64 rows of one image plus a 12-row redundantly-computed halo band, so
  NO halo exchange is needed: a 3x3 conv propagates the fake-boundary error
  1 row/step and 12 > 10 steps, so owned rows stay exact.
- Bottom-half cores store their rows REVERSED (and receive dy-flipped tap
  weights) so every core's owned region is local rows 0..63 == row-tiles
  0..15: the SPMD program is identical on all cores, and the BN AllReduce
  launches right after tile 15 while halo tiles 16..18 still compute.
- conv0/conv1/concat/fc0 are algebraically fused into 9 "tap" matmuls:
  h = sum_taps shift_tap(state) @ Wc[tap] + bias_total. Reflect padding is
  materialized as pad rows/cols of the fp16 state buffer whose upper 64
  partitions hold the +1-row-shifted image, so the 9 taps collapse to
  3 K=128 matmuls (dy in {-1,0} pairs) + 3 K=64 matmuls (dy=+1) per
  hidden-half, all reading strided windows of the same buffer (no copies).
- BatchNorm batch stats: per-core partial sums over owned pixels + a tiny
  [128,4] AllReduce per step. The BN affine is algebraically folded away:
      dx = ((h + sh2) * mask) @ (diag(scl) @ fc1)        sh2 = shift/scl
         = (h * mask) @ fc1p + mask @ G,
  with fc1p = diag(scl) @ fc1 and G = diag(sh2) @ fc1p computed per step in
  four tiny ops, so per pixel only a plain fp16 multiply remains on DVE.
- Dropout/fire masks come from jax threefry with fixed seed(42) -> input-
  independent; precomputed on host, combined, streamed as fp16 {0,1}; the
  1/(1-p) dropout scale is folded into fc1_w.
- State lives entirely in SBUF as fp16 for the whole 10-step rollout in a
  single kernel launch (in-place residual updates; lower+upper written from
  the same PSUM tile so both copies stay bit-identical).
"""

import sys

for _p in ("/opt/trn_rl_repo", "/root/.axon_site/_ro/trn_rl_repo"):
    if _p not in sys.path:
        sys.path.append(_p)

import numpy as np

C_N = 64
HIDDEN = 256
FIRE_RATE = 0.5
DROP = 0.25
EPS = 1e-5
SLOPE = 0.01

B, H, W = 4, 128, 128
NCORES = 8
HALO = 12                  # >= steps, multiple of 4
ROWS = 64 + HALO           # 76 computed rows per core
TILES = ROWS // 4          # 19 row-tiles of 4 rows (512 pixels)
OWNED_TILES = 16           # local tiles 0..15 are the owned 64 rows
WP = W + 2                 # reflect-padded width
BROWS = ROWS + 2           # + top/bottom pad rows
NPIX_TOT = float(B * H * W)

_CACHE = {}


def _build_program(steps):
    import concourse.bacc as bacc
    import concourse.mybir as mybir
    from concourse import tile

    f32 = mybir.dt.float32
    f16 = mybir.dt.float16
    AT = mybir.ActivationFunctionType
    ALU = mybir.AluOpType

    nc = bacc.Bacc("TRN2", target_bir_lowering=False, debug=False,
                   enable_asserts=True, num_devices=NCORES)

    mirror0 = nc.dram_tensor("mirror0", [128, BROWS, WP], f16, kind="ExternalInput")
    masks = nc.dram_tensor("masks", [steps, 2, TILES, 128, 512], f16, kind="ExternalInput")
    wtap = nc.dram_tensor("wtap", [128, 1536], f16, kind="ExternalInput")
    fc1w = nc.dram_tensor("fc1w", [128, 128], f16, kind="ExternalInput")
    consts = nc.dram_tensor("consts", [128, 8], f32, kind="ExternalInput")
    out_t = nc.dram_tensor("out", [2, ROWS, W], f16, kind="ExternalOutput")

    with tile.TileContext(nc, num_cores=NCORES) as tc:
        with tc.tile_pool(name="const", bufs=1) as cpool, \
             tc.tile_pool(name="work", bufs=3) as wpool, \
             tc.tile_pool(name="mpool", bufs=16) as mpool, \
             tc.tile_pool(name="small", bufs=2) as spool, \
             tc.tile_pool(name="psum", bufs=2, space="PSUM") as ppool, \
             tc.tile_pool(name="dram", bufs=2, space="DRAM") as dpool:

            w_sb = cpool.tile([128, 1536], f16)
            fc1_sb = cpool.tile([128, 128], f16)
            c_sb = cpool.tile([128, 8], f32)
            mirror = cpool.tile([128, BROWS, WP], f16)
            h_sb = cpool.tile([128, 2 * TILES * 512], f16)
            stats = cpool.tile([128, 4 * OWNED_TILES], f32)

            nc.sync.dma_start(w_sb[:], wtap[:])
            nc.sync.dma_start(fc1_sb[:], fc1w[:])
            nc.sync.dma_start(c_sb[:], consts[:])
            nc.sync.dma_start(mirror[:], mirror0[:])

            for s in range(steps):

                def pass1_tile(t, accum):
                    for hh in range(2):
                        hp = ppool.tile([128, 512], f32, tag="hp")
                        for i in range(3):          # dx = -1, 0, 1
                            nc.tensor.matmul(      # dy=-1 (lower) + dy=0 (upper)
                                hp[:],
                                w_sb[:, (i * 2 + hh) * 128:(i * 2 + hh + 1) * 128],
                                mirror[:, 4 * t: 4 * t + 4, i: i + 128],
                                start=(i == 0), stop=False)
                        for i in range(3):
                            # dy=+1 via upper half; lower weight rows are zero
                            # so K stays 128 (full array keeps HAM at K=8/8)
                            nc.tensor.matmul(
                                hp[:],
                                w_sb[:, 768 + (i * 2 + hh) * 128: 768 + (i * 2 + hh + 1) * 128],
                                mirror[:, 4 * t + 1: 4 * t + 5, i: i + 128],
                                start=False, stop=(i == 2))
                        hsl = h_sb[:, (t * 2 + hh) * 512:(t * 2 + hh + 1) * 512]
                        if accum:
                            nc.scalar.activation(
                                hsl, hp[:], AT.Lrelu, bias=c_sb[:, hh:hh + 1],
                                scale=1.0, alpha=SLOPE,
                                accum_out=stats[:, hh * OWNED_TILES + t: hh * OWNED_TILES + t + 1])
                            sq = wpool.tile([128, 512], f16, tag="sq")
                            nc.scalar.activation(
                                sq[:], hsl, AT.Square,
                                accum_out=stats[:, (2 + hh) * OWNED_TILES + t: (2 + hh) * OWNED_TILES + t + 1])
                        else:
                            nc.scalar.activation(hsl, hp[:], AT.Lrelu,
                                                 bias=c_sb[:, hh:hh + 1],
                                                 scale=1.0, alpha=SLOPE)

                # ---- pass 1 on owned tiles, then AR, then halo tiles ----
                for t in range(OWNED_TILES):
                    pass1_tile(t, True)

                loc4 = spool.tile([128, 4], f32, tag="loc4")
                nc.vector.tensor_reduce(
                    loc4[:], stats[:].rearrange("p (g t) -> p g t", t=OWNED_TILES),
                    axis=mybir.AxisListType.X, op=ALU.add)
                cin = dpool.tile([128, 4], f32, tag="cin")
                cout = dpool.tile([128, 4], f32, tag="cout")
                nc.sync.dma_start(cin[:], loc4[:])
                nc.gpsimd.collective_compute(
                    "AllReduce", ALU.add,
                    replica_groups=[list(range(NCORES))],
                    ins=[cin.opt()], outs=[cout.opt()])
                gs = spool.tile([128, 4], f32, tag="gs")
                nc.sync.dma_start(gs[:], cout[:])

                for t in range(OWNED_TILES, TILES):
                    pass1_tile(t, False)

                # ---- BN coefficients (all DVE except the Sqrt) ----
                mug = spool.tile([128, 4], f32, tag="mug")
                nc.vector.tensor_scalar_mul(mug[:], gs[:], 1.0 / NPIX_TOT)
                musq = spool.tile([128, 2], f32, tag="musq")
                nc.vector.tensor_mul(musq[:], mug[:, 0:2], mug[:, 0:2])
                var = spool.tile([128, 2], f32, tag="var")
                nc.vector.tensor_sub(var[:], mug[:, 2:4], musq[:])
                std = spool.tile([128, 2], f32, tag="std")
                nc.scalar.activation(std[:], var[:], AT.Sqrt, bias=c_sb[:, 6:7])
                inv = spool.tile([128, 2], f32, tag="inv")
                nc.vector.reciprocal(inv[:], std[:])
                scl = spool.tile([128, 2], f32, tag="scl")
                nc.vector.tensor_mul(scl[:], c_sb[:, 2:4], inv[:])
                rb = spool.tile([128, 2], f32, tag="rb")
                nc.vector.reciprocal(rb[:], scl[:])
                sh2a = spool.tile([128, 2], f32, tag="sh2a")
                nc.vector.tensor_mul(sh2a[:], c_sb[:, 4:6], rb[:])
                sh2 = spool.tile([128, 2], f32, tag="sh2")
                nc.vector.tensor_sub(sh2[:], sh2a[:], mug[:, 0:2])
                fc1p = spool.tile([128, 128], f16, tag="fc1p")
                nc.vector.tensor_scalar_mul(fc1p[:, 0:64], fc1_sb[:, 0:64], scl[:, 0:1])
                nc.vector.tensor_scalar_mul(fc1p[:, 64:128], fc1_sb[:, 64:128], scl[:, 1:2])
                gmat = spool.tile([128, 128], f16, tag="gmat")
                nc.vector.tensor_scalar_mul(gmat[:, 0:64], fc1p[:, 0:64], sh2[:, 0:1])
                nc.vector.tensor_scalar_mul(gmat[:, 64:128], fc1p[:, 64:128], sh2[:, 1:2])

                # ---- pass 2: dx = (h*mask) @ fc1p + mask @ G ; state += dx ----
                for t in range(TILES):
                    dxp = ppool.tile([C_N, 512], f32, tag="dxp")
                    for hh in range(2):
                        m_t = mpool.tile([128, 512], f16, tag="m")
                        nc.sync.dma_start(m_t[:], masks[s, hh, t])
                        hf = wpool.tile([128, 512], f16, tag="hf")
                        nc.vector.tensor_mul(
                            hf[:], h_sb[:, (t * 2 + hh) * 512:(t * 2 + hh + 1) * 512], m_t[:])
                        nc.tensor.matmul(dxp[:], fc1p[:, hh * 64:(hh + 1) * 64], hf[:],
                                         start=(hh == 0), stop=False)
                        nc.tensor.matmul(dxp[:], gmat[:, hh * 64:(hh + 1) * 64], m_t[:],
                                         start=False, stop=(hh == 1))
                    # upper first (reads the pre-update lower rows), then lower
                    nc.vector.tensor_add(
                        out=mirror[C_N:128, 4 * t: 4 * t + 4, 1:129],
                        in0=dxp[:].rearrange("p (a b) -> p a b", b=128),
                        in1=mirror[0:C_N, 1 + 4 * t: 5 + 4 * t, 1:129])
                    nc.vector.tensor_add(
                        out=mirror[0:C_N, 1 + 4 * t: 5 + 4 * t, 1:129],
                        in0=dxp[:].rearrange("p (a b) -> p a b", b=128),
                        in1=mirror[0:C_N, 1 + 4 * t: 5 + 4 * t, 1:129])

                # ---- reflect pads ----
                nc.gpsimd.tensor_copy(mirror[0:C_N, 0:1, 1:129], mirror[0:C_N, 2:3, 1:129])
                nc.gpsimd.tensor_copy(mirror[0:C_N, BROWS - 1:BROWS, 1:129],
                                      mirror[0:C_N, BROWS - 3:BROWS - 2, 1:129])
                nc.gpsimd.tensor_copy(mirror[C_N:128, BROWS - 2:BROWS - 1, 1:129],
                                      mirror[C_N:128, BROWS - 4:BROWS - 3, 1:129])
                nc.gpsimd.tensor_copy(mirror[:, :, 0:1], mirror[:, :, 2:3])
                nc.gpsimd.tensor_copy(mirror[:, :, WP - 1:WP], mirror[:, :, WP - 3:WP - 2])

            nc.sync.dma_start(out_t[:], mirror[0:2, 1:BROWS - 1, 1:WP - 1])

    nc.compile()
    return nc


def _host_masks(steps):
    """Combined dropout-keep & fire masks, exactly matching the reference's
    threefry stream. Input-independent (seed 42)."""
    import jax

    cpu = jax.devices("cpu")[0]
    with jax.default_device(cpu):
        keys = jax.random.split(jax.random.key(42), steps)
        out = np.empty((steps, B, H, W, HIDDEN), np.uint8)
        for s in range(steps):
            k_drop, k_fire = jax.random.split(keys[s])
            keep = jax.random.bernoulli(k_drop, 1.0 - DROP, (B, H, W, HIDDEN))
            fire = jax.random.uniform(k_fire, (B, H, W, 1)) > FIRE_RATE
            out[s] = np.asarray(keep & fire, np.uint8)
    return out


def _prep_inputs(inputs, steps):
    x = np.asarray(inputs["x"], np.float32)
    t = np.asarray(inputs["t"], np.float32)
    p0w = np.asarray(inputs["p0_w"], np.float64)
    p0b = np.asarray(inputs["p0_b"], np.float64)
    p1w = np.asarray(inputs["p1_w"], np.float64)
    p1b = np.asarray(inputs["p1_b"], np.float64)
    fc0w = np.asarray(inputs["fc0_w"], np.float64)
    fc0b = np.asarray(inputs["fc0_b"], np.float64)
    fc1w = np.asarray(inputs["fc1_w"], np.float64)
    gamma = np.asarray(inputs["bn_gamma"], np.float32)
    beta = np.asarray(inputs["bn_beta"], np.float32)

    # fused conv+fc0 tap weights, [3(dy),3(dx),64,256]
    Wc = np.zeros((3, 3, C_N, HIDDEN), np.float32)
    for r in range(3):
        for c in range(3):
            wc = p0w[r, c] @ fc0w[C_N:2 * C_N] + p1w[r, c] @ fc0w[2 * C_N:]
            if r == 1 and c == 1:
                wc = wc + fc0w[0:C_N]
            Wc[r, c] = wc.astype(np.float32)
    bias_total = (p0b @ fc0w[C_N:2 * C_N] + p1b @ fc0w[2 * C_N:] + fc0b).astype(np.float32)

    def build_wtap(flip):
        wt = np.zeros((128, 1536), np.float16)
        for i in range(3):          # dx index
            for hh in range(2):
                wm1 = Wc[2 if flip else 0, i]     # local dy=-1
                w0 = Wc[1, i]                     # local dy=0
                wp1 = Wc[0 if flip else 2, i]     # local dy=+1
                col = (i * 2 + hh) * 128
                wt[0:C_N, col:col + 128] = wm1[:, hh * 128:(hh + 1) * 128]
                wt[C_N:128, col:col + 128] = w0[:, hh * 128:(hh + 1) * 128]
                scol = 768 + (i * 2 + hh) * 128
                wt[C_N:128, scol:scol + 128] = wp1[:, hh * 128:(hh + 1) * 128]
        return wt

    wtap_by_flip = [build_wtap(False), build_wtap(True)]

    fc1s = fc1w / (1.0 - DROP)
    fc1_host = np.zeros((128, 128), np.float16)
    for hh in range(2):
        fc1_host[:, hh * 64:(hh + 1) * 64] = fc1s[hh * 128:(hh + 1) * 128, :].astype(np.float16)

    consts = np.zeros((128, 8), np.float32)
    consts[:, 0] = bias_total[0:128]
    consts[:, 1] = bias_total[128:256]
    consts[:, 2] = gamma[0:128]
    consts[:, 3] = gamma[128:256]
    consts[:, 4] = beta[0:128]
    consts[:, 5] = beta[128:256]
    consts[:, 6] = EPS
    consts[:, 7] = 1.0 / NPIX_TOT

    # seed state [B,H,W,C]
    st = np.zeros((B, H, W, C_N), np.float32)
    st[..., 1] = x[:, 0]
    lin = np.linspace(0.0, 1.0, H, dtype=np.float32)
    st[..., C_N - 2] = (lin[:, None] + lin[None, :]) * 0.5
    st[..., C_N - 1] = t[0]

    keep_eff = _host_masks(steps)

    in_maps = []
    for c in range(NCORES):
        b, half = c // 2, c % 2
        flip = half == 1

        if not flip:
            sl = st[b, 0:ROWS]                       # [76,128,64] local==global
            msl = keep_eff[:, b, 0:ROWS]
        else:
            sl = st[b, H - ROWS:H][::-1]             # local i = global 127-i
            msl = keep_eff[:, b, H - ROWS:H][:, ::-1]

        p = np.concatenate([sl[1:2], sl, sl[ROWS - 2:ROWS - 1]], axis=0)
        p = np.concatenate([p[:, 1:2], p, p[:, W - 2:W - 1]], axis=1)  # [78,130,64]
        low = p.transpose(2, 0, 1).astype(np.float16)                  # [64,78,130]
        up = np.zeros_like(low)
        up[:, 0:BROWS - 1] = low[:, 1:BROWS]
        mirror0 = np.ascontiguousarray(np.concatenate([low, up], axis=0))

        m = msl.reshape(steps, TILES, 4, W, 2, 128)
        m = np.ascontiguousarray(
            m.transpose(0, 4, 1, 5, 2, 3)).reshape(steps, 2, TILES, 128, 512).astype(np.float16)

        in_maps.append({
            "mirror0": mirror0,
            "masks": m,
            "wtap": wtap_by_flip[flip],
            "fc1w": fc1_host,
            "consts": consts,
        })
    return in_maps


def _run(inputs, trace=False, **kw):
    from concourse.bass_utils import run_bass_kernel_spmd

    steps = int(np.asarray(inputs["steps"]))
    assert HALO >= steps, f"halo {HALO} < steps {steps}"
    if steps not in _CACHE:
        _CACHE[steps] = _build_program(steps)
    nc = _CACHE[steps]
    in_maps = _prep_inputs(inputs, steps)
    res = run_bass_kernel_spmd(nc, in_maps, core_ids=list(range(NCORES)),
                               trace=trace, **kw)

    out0 = np.zeros((B, 1, H, W), np.float32)
    out1 = np.zeros((B, H, W), np.float32)
    for c in range(NCORES):
        b, half = c // 2, c % 2
        r = res.results[c]["out"][:, 0:64, :].astype(np.float32)
        if half == 1:
            r = r[:, ::-1, :]
        rows = slice(half * 64, (half + 1) * 64)
        out0[b, 0, rows, :] = r[0]
        out1[b, rows, :] = r[1]
    return (out0, out1), res


def kernel(**inputs):
    (out0, out1), _ = _run(inputs)
    return out0, out1
